# revision 9
# baseline (speedup 1.0000x reference)
"""Trainium2 Bass kernel for nn_Attention_24215025615017.

8-head spatial attention block (1x1-conv QKV projections with folded BatchNorm,
transposed-softmax attention, exact GELU, output 1x1 conv with folded BN).
Data-parallel over batch: B=32 sharded as 4 batches on each of 8 NeuronCores.

Self-contained: hardcodes shapes/sharding; builds + caches one SPMD Bacc graph.
"""

import sys
import numpy as np

if '/opt/trn_rl_repo' not in sys.path:
    sys.path.insert(0, '/opt/trn_rl_repo')
_a = sys.modules.get('antenv')
if _a is not None and '_ro' in getattr(_a, '__file__', ''):
    # purge the read-only copy so antenv resolves to /opt/trn_rl_repo
    for _m in list(sys.modules):
        if _m == 'antenv' or _m.startswith('antenv.'):
            del sys.modules[_m]

import ml_dtypes

EPS = 1e-5
HEADS = 8
DK = 32
DV = 64
B_TOT = 32
N_CORES = 8
B_LOC = B_TOT // N_CORES  # 4 batches per core
C_IN = 256                # input channels
C_V = 512                 # v channels (h*dv)
N = 1024                  # pixels (32*32)
VSTRIDE = DV + 1          # v_aug block: 64 data cols + ones col

_cache = {}


def _build():
    import concourse.bass as bass
    import concourse.tile as tile
    from concourse import bacc, mybir

    f32 = mybir.dt.float32
    bf16 = mybir.dt.bfloat16
    Exp = mybir.ActivationFunctionType.Exp
    Gelu = mybir.ActivationFunctionType.Gelu
    mult = mybir.AluOpType.add  # placeholder, reassigned below
    mult = mybir.AluOpType.mult
    add = mybir.AluOpType.add

    nc = bacc.Bacc("TRN2", target_bir_lowering=False, debug=False,
                   num_devices=N_CORES)

    x_ext = nc.declare_dram_parameter("x", [B_LOC, C_IN, N], f32, isOutput=False)
    wqT_ext = nc.declare_dram_parameter("wqT", [2, 128, 256], bf16, isOutput=False)
    wkT_ext = nc.declare_dram_parameter("wkT", [2, 128, 256], bf16, isOutput=False)
    wvT_ext = nc.declare_dram_parameter("wvT", [2, 128, 512], bf16, isOutput=False)
    woT_ext = nc.declare_dram_parameter("woT", [4, 128, 256], bf16, isOutput=False)
    shq_ext = nc.declare_dram_parameter("shq", [128, 2], f32, isOutput=False)
    shk_ext = nc.declare_dram_parameter("shk", [128, 2], f32, isOutput=False)
    shv_ext = nc.declare_dram_parameter("shv", [1, 512], bf16, isOutput=False)
    ones_ext = nc.declare_dram_parameter("onesr", [1, 128], bf16, isOutput=False)
    bo_ext = nc.declare_dram_parameter("bo", [128, 2], f32, isOutput=False)
    out_ext = nc.declare_dram_parameter("out", [B_LOC, C_IN, N], f32, isOutput=True)

    from contextlib import ExitStack
    with tile.TileContext(nc) as tc, ExitStack() as ctx:
        consts = ctx.enter_context(tc.tile_pool(name="consts", bufs=1))
        vpool = ctx.enter_context(tc.tile_pool(name="vaug", bufs=1))
        xfp = ctx.enter_context(tc.tile_pool(name="xf", bufs=2))
        xbp = ctx.enter_context(tc.tile_pool(name="xb", bufs=2))
        qkp = ctx.enter_context(tc.tile_pool(name="qk", bufs=2))
        pp = ctx.enter_context(tc.tile_pool(name="pp", bufs=2))
        gp = ctx.enter_context(tc.tile_pool(name="gp", bufs=3))
        gbfp = ctx.enter_context(tc.tile_pool(name="gbf", bufs=2))
        lrp = ctx.enter_context(tc.tile_pool(name="lr", bufs=4))
        rbp = ctx.enter_context(tc.tile_pool(name="rb", bufs=4))
        osp = ctx.enter_context(tc.tile_pool(name="os", bufs=4))
        ps_big = ctx.enter_context(tc.tile_pool(name="psb", bufs=1, space="PSUM"))
        ps_av = ctx.enter_context(tc.tile_pool(name="psav", bufs=2, space="PSUM"))

        # ---- load constants ----
        wq_sb = [consts.tile([128, 256], bf16, tag=f"wq{t}", name=f"wq{t}") for t in range(2)]
        wk_sb = [consts.tile([128, 256], bf16, tag=f"wk{t}", name=f"wk{t}") for t in range(2)]
        wv_sb = [consts.tile([128, 512], bf16, tag=f"wv{t}", name=f"wv{t}") for t in range(2)]
        wo_sb = [consts.tile([128, 256], bf16, tag=f"wo{t}", name=f"wo{t}") for t in range(4)]
        shq_sb = consts.tile([128, 2], f32, tag="shq", name="shq")
        shk_sb = consts.tile([128, 2], f32, tag="shk", name="shk")
        shv_sb = consts.tile([1, 512], bf16, tag="shv", name="shv")
        ones_sb = consts.tile([1, 128], bf16, tag="ones", name="ones")
        bo_sb = consts.tile([128, 2], f32, tag="bo", name="bo")
        for t in range(2):
            nc.gpsimd.dma_start(out=wq_sb[t][:], in_=wqT_ext.ap()[t])
            nc.gpsimd.dma_start(out=wk_sb[t][:], in_=wkT_ext.ap()[t])
            nc.gpsimd.dma_start(out=wv_sb[t][:], in_=wvT_ext.ap()[t])
        for t in range(4):
            nc.gpsimd.dma_start(out=wo_sb[t][:], in_=woT_ext.ap()[t])
        nc.gpsimd.dma_start(out=shq_sb[:], in_=shq_ext.ap()[:])
        nc.gpsimd.dma_start(out=shk_sb[:], in_=shk_ext.ap()[:])
        nc.gpsimd.dma_start(out=shv_sb[:], in_=shv_ext.ap()[:])
        nc.gpsimd.dma_start(out=ones_sb[:], in_=ones_ext.ap()[:])
        nc.gpsimd.dma_start(out=bo_sb[:], in_=bo_ext.ap()[:])

        # two persistent v_aug buffers (ones columns memset once, data columns
        # rewritten per batch; av reads l from the ones column product)
        vaug = [vpool.tile([128, 64 * VSTRIDE], bf16, tag=f"vaug{i}", name=f"vaug{i}") for i in range(2)]
        nc.vector.memset(vaug[0][:], 1.0)
        nc.vector.memset(vaug[1][:], 1.0)

        for b in range(B_LOC):
            va = vaug[b % 2]
            # ---- load + cast x ----
            xf = xfp.tile([128, 2048], f32, tag="xf", name="xf")
            nc.gpsimd.dma_start(
                out=xf[:].rearrange("p (t n) -> p t n", t=2),
                in_=x_ext.ap()[b].rearrange("(t p) n -> p t n", p=128))
            xb = xbp.tile([128, 2048], bf16, tag="xb", name="xb")
            nc.vector.tensor_copy(xb[:], xf[:])

            # ---- Q/K projections (BN folded into weights; bias via evict) ----
            q_sb = [qkp.tile([128, N], bf16, tag=f"q{t}", name=f"q{t}") for t in range(2)]
            k_sb = [qkp.tile([128, N], bf16, tag=f"k{t}", name=f"k{t}") for t in range(2)]
            for (w_sb, sh_sb, dst) in ((wq_sb, shq_sb, q_sb), (wk_sb, shk_sb, k_sb)):
                ps = ps_big.tile([128, 2048], f32, tag="pd", name="pd")
                for t in range(2):
                    for ih in range(2):
                        for kt in range(2):
                            nc.tensor.matmul(
                                ps[:, t * 1024 + ih * 512:t * 1024 + ih * 512 + 512],
                                w_sb[kt][:, t * 128:(t + 1) * 128],
                                xb[:, kt * 1024 + ih * 512:kt * 1024 + ih * 512 + 512],
                                start=(kt == 0), stop=(kt == 1))
                for t in range(2):
                    nc.vector.tensor_scalar(
                        dst[t][:], ps[:, t * 1024:(t + 1) * 1024],
                        sh_sb[:, t:t + 1], None, add)

            # ---- V projection, transposed ([pixel, channel]), bias via K=1 mm ----
            for half in range(2):  # j-chunks 4 at a time
                ps = ps_big.tile([128, 2048], f32, tag="pd", name="pd")
                for cq in range(4):
                    jc = half * 4 + cq
                    for kt in range(2):
                        nc.tensor.matmul(
                            ps[:, cq * 512:(cq + 1) * 512],
                            xb[:, kt * 1024 + jc * 128:kt * 1024 + jc * 128 + 128],
                            wv_sb[kt][:],
                            start=(kt == 0), stop=False)
                    nc.tensor.matmul(
                        ps[:, cq * 512:(cq + 1) * 512],
                        ones_sb[0:1, 0:128], shv_sb[0:1, :],
                        start=False, stop=True)
                for cq in range(4):
                    jc = half * 4 + cq
                    src = ps[:, cq * 512:(cq + 1) * 512].rearrange(
                        "p (h d) -> p h d", h=8)
                    dst = va[:, jc * 8 * VSTRIDE:(jc + 1) * 8 * VSTRIDE].rearrange(
                        "p (h e) -> p h e", h=8)[:, :, 0:DV]
                    nc.vector.tensor_copy(dst, src)

            # ---- attention, head pairs on distinct PE row groups ----
            gpre = []
            for p in range(4):
                h0 = 2 * p
                t_q = h0 // 4
                offs = (32 * (h0 % 4), 32 * (h0 % 4) + 32)
                P = pp.tile([128, 2 * 8192], bf16, tag="P", name="P")
                av = [ps_av.tile([128, N], f32, tag="av", name="av") for _ in range(2)]
                for jc in range(8):
                    pd = ps_big.tile([128, 2048], f32, tag="pd", name="pd")
                    for hi in range(2):
                        off = offs[hi]
                        for ih in range(2):
                            nc.tensor.matmul(
                                pd[:, hi * 1024 + ih * 512:hi * 1024 + ih * 512 + 512],
                                k_sb[t_q][off:off + 32, jc * 128:(jc + 1) * 128],
                                q_sb[t_q][off:off + 32, ih * 512:(ih + 1) * 512],
                                start=True, stop=True,
                                tile_position=(off, 0))
                    nc.scalar.activation(
                        P[:].rearrange("p (h n) -> p h n", h=2)[:, :, jc * 1024:(jc + 1) * 1024],
                        pd[:].rearrange("p (h n) -> p h n", h=2),
                        Exp)
                    for hi in range(2):
                        h = h0 + hi
                        for ih in range(2):
                            nc.tensor.matmul(
                                av[hi][0:65, ih * 512:(ih + 1) * 512],
                                va[:, jc * 8 * VSTRIDE + h * VSTRIDE:
                                   jc * 8 * VSTRIDE + h * VSTRIDE + VSTRIDE],
                                P[:, hi * 8192 + jc * 1024 + ih * 512:
                                  hi * 8192 + jc * 1024 + ih * 512 + 512],
                                start=(jc == 0), stop=(jc == 7))
                # softmax normalizer: l row -> reciprocal -> broadcast
                g = gp.tile([128, N], bf16, tag="gpre", name="gpre")
                gpre.append(g)
                for hi in range(2):
                    rh = lrp.tile([1, N], f32, tag="rh", name="rh")
                    nc.vector.reciprocal(rh[0:1, :], av[hi][64:65, :])
                    R = rbp.tile([64, N], f32, tag="R", name="R")
                    nc.gpsimd.partition_broadcast(R[:], rh[0:1, :])
                    nc.vector.tensor_tensor(
                        g[hi * 64:hi * 64 + 64, :], av[hi][0:64, :], R[:], mult)

            # ---- gelu (batched per batch to limit ACT table switches) ----
            gbf = []
            for p in range(4):
                gt = gbfp.tile([128, N], bf16, tag=f"gbf{p}", name=f"gbf{p}")
                gbf.append(gt)
                nc.scalar.activation(gt[:], gpre[p][:], Gelu)

            # ---- output projection + folded BN bias ----
            ps_o = ps_av.tile([128, N], f32, tag="av", name="av")
            ps_o2 = ps_av.tile([128, N], f32, tag="av", name="av")
            for ot, pso in ((0, ps_o), (1, ps_o2)):
                for kt in range(4):
                    for ih in range(2):
                        nc.tensor.matmul(
                            pso[:, ih * 512:(ih + 1) * 512],
                            wo_sb[kt][:, ot * 128:(ot + 1) * 128],
                            gbf[kt][:, ih * 512:(ih + 1) * 512],
                            start=(kt == 0), stop=(kt == 3))
                osb = osp.tile([128, N], f32, tag="osb", name="osb")
                nc.vector.tensor_scalar(osb[:], pso[:, 0:N],
                                        bo_sb[:, ot:ot + 1], None, add)
                nc.gpsimd.dma_start(
                    out=out_ext.ap()[b, ot * 128:(ot + 1) * 128, :], in_=osb[:])

    nc.compile()
    return nc


def _get_nc():
    if 'nc' not in _cache:
        _cache['nc'] = _build()
    return _cache['nc']


def _fold_weights(inputs):
    """Fold BatchNorms (+ attention scale) into conv weights, host-side."""
    f8 = {k: np.asarray(v, np.float64) for k, v in inputs.items()}
    scale = DK ** -0.5

    def fold(w, g, b, m, v, extra=1.0):
        inv = g / np.sqrt(v + EPS)
        return w * inv[:, None] * extra, (b - m * inv) * extra

    wq_e, shq = fold(f8['wq'], f8['gq'], f8['bq'], f8['mq'], f8['vq'], scale)
    wk_e, shk = fold(f8['wk'], f8['gk'], f8['bk'], f8['mk'], f8['vk'])
    wv_e, shv = fold(f8['wv'], f8['gv'], f8['bv'], f8['mv'], f8['vv'])
    inv_o = f8['go'] / np.sqrt(f8['vo'] + EPS)
    wo_e = f8['wo'] * inv_o[:, None]
    bo = inv_o * f8['b_out'] + (f8['be_o'] - f8['mo'] * inv_o)

    bf = ml_dtypes.bfloat16
    return {
        'wqT': np.ascontiguousarray(wq_e.T).reshape(2, 128, 256).astype(bf),
        'wkT': np.ascontiguousarray(wk_e.T).reshape(2, 128, 256).astype(bf),
        'wvT': np.ascontiguousarray(wv_e.T).reshape(2, 128, 512).astype(bf),
        'woT': np.ascontiguousarray(wo_e.T).reshape(4, 128, 256).astype(bf),
        'shq': np.ascontiguousarray(shq.reshape(2, 128).T).astype(np.float32),
        'shk': np.ascontiguousarray(shk.reshape(2, 128).T).astype(np.float32),
        'shv': shv.reshape(1, 512).astype(bf),
        'onesr': np.ones((1, 128), bf),
        'bo': np.ascontiguousarray(bo.reshape(2, 128).T).astype(np.float32),
    }


def kernel_run(inputs, trace=False, trace_kwargs=None):
    from concourse.bass_utils import run_bass_kernel_spmd
    nc = _get_nc()
    consts = _fold_weights(inputs)
    x = np.asarray(inputs['x'], np.float32).reshape(B_TOT, C_IN, N)
    in_maps = []
    for c in range(N_CORES):
        m = dict(consts)
        m['x'] = np.ascontiguousarray(x[c * B_LOC:(c + 1) * B_LOC])
        in_maps.append(m)
    res = run_bass_kernel_spmd(nc, in_maps, core_ids=list(range(N_CORES)),
                               trace=trace, **(trace_kwargs or {}))
    out = np.concatenate([res.results[c]['out'] for c in range(N_CORES)], axis=0)
    return out.reshape(B_TOT, C_IN, 32, 32), res


def kernel(**inputs) -> np.ndarray:
    out, _ = kernel_run(inputs, trace=False)
    return out


# revision 20
# speedup vs baseline: 1.1786x; 1.1786x over previous
"""Trainium2 Bass kernel for nn_Attention_24215025615017.

8-head spatial attention block (1x1-conv QKV projections with folded BatchNorm,
transposed-softmax attention, exact GELU, output 1x1 conv with folded BN).
Data-parallel over batch: B=32 sharded as 4 batches on each of 8 NeuronCores.

Self-contained: hardcodes shapes/sharding; builds + caches one SPMD Bacc graph.
"""

import sys
import numpy as np

if '/opt/trn_rl_repo' not in sys.path:
    sys.path.insert(0, '/opt/trn_rl_repo')
_a = sys.modules.get('antenv')
if _a is not None and '_ro' in getattr(_a, '__file__', ''):
    # purge the read-only copy so antenv resolves to /opt/trn_rl_repo
    for _m in list(sys.modules):
        if _m == 'antenv' or _m.startswith('antenv.'):
            del sys.modules[_m]

import ml_dtypes

EPS = 1e-5
HEADS = 8
DK = 32
DV = 64
B_TOT = 32
N_CORES = 8
B_LOC = B_TOT // N_CORES  # 4 batches per core
C_IN = 256                # input channels
C_V = 512                 # v channels (h*dv)
N = 1024                  # pixels (32*32)
VSTRIDE = DV + 1          # v_aug block: 64 data cols + ones col

_cache = {}


def _build():
    import concourse.bass as bass
    import concourse.tile as tile
    from concourse import bacc, mybir

    f32 = mybir.dt.float32
    bf16 = mybir.dt.bfloat16
    Exp = mybir.ActivationFunctionType.Exp
    Tanh = mybir.ActivationFunctionType.Tanh
    mult = mybir.AluOpType.add  # placeholder, reassigned below
    mult = mybir.AluOpType.mult
    add = mybir.AluOpType.add

    nc = bacc.Bacc("TRN2", target_bir_lowering=False, debug=False,
                   num_devices=N_CORES)

    x_ext = nc.declare_dram_parameter("x", [B_LOC, C_IN, N], f32, isOutput=False)
    wqT_ext = nc.declare_dram_parameter("wqT", [2, 128, 256], bf16, isOutput=False)
    wkT_ext = nc.declare_dram_parameter("wkT", [2, 128, 256], bf16, isOutput=False)
    wvT_ext = nc.declare_dram_parameter("wvT", [2, 128, 512], bf16, isOutput=False)
    woT_ext = nc.declare_dram_parameter("woT", [4, 128, 256], bf16, isOutput=False)
    shq_ext = nc.declare_dram_parameter("shq", [128, 2], f32, isOutput=False)
    shk_ext = nc.declare_dram_parameter("shk", [128, 2], f32, isOutput=False)
    shv_ext = nc.declare_dram_parameter("shv", [1, 512], bf16, isOutput=False)
    ones_ext = nc.declare_dram_parameter("onesr", [1, 128], bf16, isOutput=False)
    bo_ext = nc.declare_dram_parameter("bo", [128, 2], f32, isOutput=False)
    out_ext = nc.declare_dram_parameter("out", [B_LOC, C_IN, N], f32, isOutput=True)

    from contextlib import ExitStack
    with tile.TileContext(nc) as tc, ExitStack() as ctx:
        consts = ctx.enter_context(tc.tile_pool(name="consts", bufs=1))
        vpool = ctx.enter_context(tc.tile_pool(name="vaug", bufs=1))
        xfp = ctx.enter_context(tc.tile_pool(name="xf", bufs=2))
        xbp = ctx.enter_context(tc.tile_pool(name="xb", bufs=2))
        qkp = ctx.enter_context(tc.tile_pool(name="qk", bufs=2))
        pp = ctx.enter_context(tc.tile_pool(name="pp", bufs=2))
        gp = ctx.enter_context(tc.tile_pool(name="gp", bufs=3))
        gbfp = ctx.enter_context(tc.tile_pool(name="gbf", bufs=1))
        lrp = ctx.enter_context(tc.tile_pool(name="lr", bufs=2))
        rbp = ctx.enter_context(tc.tile_pool(name="rb", bufs=2))
        osp = ctx.enter_context(tc.tile_pool(name="os", bufs=2))
        ps_big = ctx.enter_context(tc.tile_pool(name="psb", bufs=1, space="PSUM"))
        ps_av = ctx.enter_context(tc.tile_pool(name="psav", bufs=2, space="PSUM"))

        # ---- load constants ----
        wq_sb = [consts.tile([128, 256], bf16, tag=f"wq{t}", name=f"wq{t}") for t in range(2)]
        wk_sb = [consts.tile([128, 256], bf16, tag=f"wk{t}", name=f"wk{t}") for t in range(2)]
        wv_sb = [consts.tile([128, 512], bf16, tag=f"wv{t}", name=f"wv{t}") for t in range(2)]
        wo_sb = [consts.tile([128, 256], bf16, tag=f"wo{t}", name=f"wo{t}") for t in range(4)]
        shq_sb = consts.tile([128, 2], f32, tag="shq", name="shq")
        shk_sb = consts.tile([128, 2], f32, tag="shk", name="shk")
        shv_sb = consts.tile([1, 512], bf16, tag="shv", name="shv")
        ones_sb = consts.tile([1, 128], bf16, tag="ones", name="ones")
        bo_sb = consts.tile([128, 2], f32, tag="bo", name="bo")
        for t in range(2):
            nc.gpsimd.dma_start(out=wq_sb[t][:], in_=wqT_ext.ap()[t])
            nc.gpsimd.dma_start(out=wk_sb[t][:], in_=wkT_ext.ap()[t])
            nc.gpsimd.dma_start(out=wv_sb[t][:], in_=wvT_ext.ap()[t])
        for t in range(4):
            nc.gpsimd.dma_start(out=wo_sb[t][:], in_=woT_ext.ap()[t])
        nc.gpsimd.dma_start(out=shq_sb[:], in_=shq_ext.ap()[:])
        nc.gpsimd.dma_start(out=shk_sb[:], in_=shk_ext.ap()[:])
        nc.gpsimd.dma_start(out=shv_sb[:], in_=shv_ext.ap()[:])
        nc.gpsimd.dma_start(out=ones_sb[:], in_=ones_ext.ap()[:])
        nc.gpsimd.dma_start(out=bo_sb[:], in_=bo_ext.ap()[:])

        # two persistent v_aug buffers (ones columns memset once, data columns
        # rewritten per batch; av reads l from the ones column product)
        vaug = [vpool.tile([128, 64 * VSTRIDE], bf16, tag=f"vaug{i}", name=f"vaug{i}") for i in range(2)]
        nc.vector.memset(vaug[0][:], 1.0)
        nc.vector.memset(vaug[1][:], 1.0)

        for b in range(B_LOC):
            va = vaug[b % 2]
            # ---- load + cast x ----
            xf = xfp.tile([128, 2048], f32, tag="xf", name="xf")
            nc.gpsimd.dma_start(
                out=xf[:].rearrange("p (t n) -> p t n", t=2),
                in_=x_ext.ap()[b].rearrange("(t p) n -> p t n", p=128))
            xb = xbp.tile([128, 2048], bf16, tag="xb", name="xb")
            nc.vector.tensor_copy(xb[:], xf[:])

            # ---- Q/K projections (BN folded into weights; bias via evict) ----
            q_sb = [qkp.tile([128, N], bf16, tag=f"q{t}", name=f"q{t}") for t in range(2)]
            k_sb = [qkp.tile([128, N], bf16, tag=f"k{t}", name=f"k{t}") for t in range(2)]
            for (w_sb, sh_sb, dst) in ((wq_sb, shq_sb, q_sb), (wk_sb, shk_sb, k_sb)):
                ps = ps_big.tile([128, 2048], f32, tag="pd", name="pd")
                for t in range(2):
                    for ih in range(2):
                        for kt in range(2):
                            nc.tensor.matmul(
                                ps[:, t * 1024 + ih * 512:t * 1024 + ih * 512 + 512],
                                w_sb[kt][:, t * 128:(t + 1) * 128],
                                xb[:, kt * 1024 + ih * 512:kt * 1024 + ih * 512 + 512],
                                start=(kt == 0), stop=(kt == 1))
                for t in range(2):
                    nc.vector.tensor_scalar(
                        dst[t][:], ps[:, t * 1024:(t + 1) * 1024],
                        sh_sb[:, t:t + 1], None, add)

            # ---- V projection, transposed ([pixel, channel]), bias via K=1 mm ----
            for half in range(2):  # j-chunks 4 at a time
                ps = ps_big.tile([128, 2048], f32, tag="pd", name="pd")
                for cq in range(4):
                    jc = half * 4 + cq
                    for kt in range(2):
                        nc.tensor.matmul(
                            ps[:, cq * 512:(cq + 1) * 512],
                            xb[:, kt * 1024 + jc * 128:kt * 1024 + jc * 128 + 128],
                            wv_sb[kt][:],
                            start=(kt == 0), stop=False)
                    nc.tensor.matmul(
                        ps[:, cq * 512:(cq + 1) * 512],
                        ones_sb[0:1, 0:128], shv_sb[0:1, :],
                        start=False, stop=True)
                for cq in range(4):
                    jc = half * 4 + cq
                    src = ps[:, cq * 512:(cq + 1) * 512].rearrange(
                        "p (h d) -> p h d", h=8)
                    dst = va[:, jc * 8 * VSTRIDE:(jc + 1) * 8 * VSTRIDE].rearrange(
                        "p (h e) -> p h e", h=8)[:, :, 0:DV]
                    nc.vector.tensor_copy(dst, src)

            # ---- attention, head pairs on distinct PE row groups ----
            gbf = []
            for p in range(4):
                h0 = 2 * p
                t_q = h0 // 4
                offs = (32 * (h0 % 4), 32 * (h0 % 4) + 32)
                P = pp.tile([128, 2 * 8192], bf16, tag="P", name="P")
                av = [ps_av.tile([128, N], f32, tag="av", name="av") for _ in range(2)]
                for jc in range(8):
                    pd = ps_big.tile([128, 2048], f32, tag="pd", name="pd")
                    for hi in range(2):
                        off = offs[hi]
                        for ih in range(2):
                            nc.tensor.matmul(
                                pd[:, hi * 1024 + ih * 512:hi * 1024 + ih * 512 + 512],
                                k_sb[t_q][off:off + 32, jc * 128:(jc + 1) * 128],
                                q_sb[t_q][off:off + 32, ih * 512:(ih + 1) * 512],
                                start=True, stop=True,
                                tile_position=(off, 0))
                    nc.scalar.activation(
                        P[:].rearrange("p (h n) -> p h n", h=2)[:, :, jc * 1024:(jc + 1) * 1024],
                        pd[:].rearrange("p (h n) -> p h n", h=2),
                        Exp)
                    for hi in range(2):
                        h = h0 + hi
                        for ih in range(2):
                            nc.tensor.matmul(
                                av[hi][0:65, ih * 512:(ih + 1) * 512],
                                va[:, jc * 8 * VSTRIDE + h * VSTRIDE:
                                   jc * 8 * VSTRIDE + h * VSTRIDE + VSTRIDE],
                                P[:, hi * 8192 + jc * 1024 + ih * 512:
                                  hi * 8192 + jc * 1024 + ih * 512 + 512],
                                start=(jc == 0), stop=(jc == 7))
                # softmax normalizer: l row -> reciprocal -> broadcast
                g = gp.tile([128, N], bf16, tag="gpre", name="gpre")
                for hi in range(2):
                    lsb = lrp.tile([1, N], f32, tag="lsb", name="lsb")
                    nc.vector.tensor_copy(lsb[0:1, :], av[hi][64:65, :])
                    rh = lrp.tile([1, N], f32, tag="rh", name="rh")
                    # approx recip needs in/out base partitions equal
                    nc.vector.reciprocal_approx_fast(rh[0:1, :], lsb[0:1, :])
                    R = rbp.tile([64, N], f32, tag="R", name="R")
                    nc.gpsimd.partition_broadcast(R[:], rh[0:1, :])
                    nc.vector.tensor_tensor(
                        g[hi * 64:hi * 64 + 64, :], av[hi][0:64, :], R[:], mult)
                # gelu via tanh form (tanh shares the exp ACT table set;
                # the 0.5 factor is folded into the output weights):
                # gelu(x)/0.5 = x * (1 + tanh(c*x + c*0.044715*x^3))
                GC = 0.7978845608028654
                GA = GC * 0.044715
                t1 = lrp.tile([128, N], bf16, tag="t1", name="t1")
                nc.vector.scalar_tensor_tensor(t1[:], g[:], GA, g[:], mult, mult)
                t2 = lrp.tile([128, N], bf16, tag="t2", name="t2")
                nc.vector.scalar_tensor_tensor(t2[:], t1[:], GC, g[:], add, mult)
                t3 = lrp.tile([128, N], bf16, tag="t3", name="t3")
                nc.scalar.activation(t3[:], t2[:], Tanh)
                gt = gbfp.tile([128, N], bf16, tag=f"gbf{p}", name=f"gbf{p}")
                gbf.append(gt)
                nc.vector.scalar_tensor_tensor(gt[:], t3[:], 1.0, g[:], add, mult)

            # ---- output projection + folded BN bias ----
            ps_o = ps_av.tile([128, N], f32, tag="av", name="av")
            ps_o2 = ps_av.tile([128, N], f32, tag="av", name="av")
            for ot, pso in ((0, ps_o), (1, ps_o2)):
                for kt in range(4):
                    for ih in range(2):
                        nc.tensor.matmul(
                            pso[:, ih * 512:(ih + 1) * 512],
                            wo_sb[kt][:, ot * 128:(ot + 1) * 128],
                            gbf[kt][:, ih * 512:(ih + 1) * 512],
                            start=(kt == 0), stop=(kt == 3))
                osb = osp.tile([128, N], f32, tag="osb", name="osb")
                nc.vector.tensor_scalar(osb[:], pso[:, 0:N],
                                        bo_sb[:, ot:ot + 1], None, add)
                nc.gpsimd.dma_start(
                    out=out_ext.ap()[b, ot * 128:(ot + 1) * 128, :], in_=osb[:])

    nc.compile()
    return nc


def _get_nc():
    if 'nc' not in _cache:
        _cache['nc'] = _build()
    return _cache['nc']


def _fold_weights(inputs):
    """Fold BatchNorms (+ attention scale) into conv weights, host-side."""
    f8 = {k: np.asarray(v, np.float64) for k, v in inputs.items()}
    scale = DK ** -0.5

    def fold(w, g, b, m, v, extra=1.0):
        inv = g / np.sqrt(v + EPS)
        return w * inv[:, None] * extra, (b - m * inv) * extra

    wq_e, shq = fold(f8['wq'], f8['gq'], f8['bq'], f8['mq'], f8['vq'], scale)
    wk_e, shk = fold(f8['wk'], f8['gk'], f8['bk'], f8['mk'], f8['vk'])
    wv_e, shv = fold(f8['wv'], f8['gv'], f8['bv'], f8['mv'], f8['vv'])
    inv_o = f8['go'] / np.sqrt(f8['vo'] + EPS)
    # the 0.5 of the tanh-form gelu is folded in here
    wo_e = f8['wo'] * inv_o[:, None] * 0.5
    bo = inv_o * f8['b_out'] + (f8['be_o'] - f8['mo'] * inv_o)

    bf = ml_dtypes.bfloat16
    return {
        'wqT': np.ascontiguousarray(wq_e.T).reshape(2, 128, 256).astype(bf),
        'wkT': np.ascontiguousarray(wk_e.T).reshape(2, 128, 256).astype(bf),
        'wvT': np.ascontiguousarray(wv_e.T).reshape(2, 128, 512).astype(bf),
        'woT': np.ascontiguousarray(wo_e.T).reshape(4, 128, 256).astype(bf),
        'shq': np.ascontiguousarray(shq.reshape(2, 128).T).astype(np.float32),
        'shk': np.ascontiguousarray(shk.reshape(2, 128).T).astype(np.float32),
        'shv': shv.reshape(1, 512).astype(bf),
        'onesr': np.ones((1, 128), bf),
        'bo': np.ascontiguousarray(bo.reshape(2, 128).T).astype(np.float32),
    }


def kernel_run(inputs, trace=False, trace_kwargs=None):
    from concourse.bass_utils import run_bass_kernel_spmd
    nc = _get_nc()
    consts = _fold_weights(inputs)
    x = np.asarray(inputs['x'], np.float32).reshape(B_TOT, C_IN, N)
    in_maps = []
    for c in range(N_CORES):
        m = dict(consts)
        m['x'] = np.ascontiguousarray(x[c * B_LOC:(c + 1) * B_LOC])
        in_maps.append(m)
    res = run_bass_kernel_spmd(nc, in_maps, core_ids=list(range(N_CORES)),
                               trace=trace, **(trace_kwargs or {}))
    out = np.concatenate([res.results[c]['out'] for c in range(N_CORES)], axis=0)
    return out.reshape(B_TOT, C_IN, 32, 32), res


def kernel(**inputs) -> np.ndarray:
    out, _ = kernel_run(inputs, trace=False)
    return out


# revision 24
# speedup vs baseline: 1.5310x; 1.2990x over previous
"""Trainium2 Bass kernel for nn_Attention_24215025615017.

8-head spatial attention block (1x1-conv QKV projections with folded BatchNorm,
transposed-softmax attention, exact GELU, output 1x1 conv with folded BN).
Data-parallel over batch: B=32 sharded as 4 batches on each of 8 NeuronCores.

Self-contained: hardcodes shapes/sharding; builds + caches one SPMD Bacc graph.
"""

import sys
import numpy as np

if '/opt/trn_rl_repo' not in sys.path:
    sys.path.insert(0, '/opt/trn_rl_repo')
_a = sys.modules.get('antenv')
if _a is not None and '_ro' in getattr(_a, '__file__', ''):
    # purge the read-only copy so antenv resolves to /opt/trn_rl_repo
    for _m in list(sys.modules):
        if _m == 'antenv' or _m.startswith('antenv.'):
            del sys.modules[_m]

import ml_dtypes

EPS = 1e-5
HEADS = 8
DK = 32
DV = 64
B_TOT = 32
N_CORES = 8
B_LOC = B_TOT // N_CORES  # 4 batches per core
C_IN = 256                # input channels
C_V = 512                 # v channels (h*dv)
N = 1024                  # pixels (32*32)
VSTRIDE = DV + 1          # v_aug block: 64 data cols + ones col

_cache = {}


def _build():
    import concourse.bass as bass
    import concourse.tile as tile
    from concourse import bacc, mybir

    f32 = mybir.dt.float32
    bf16 = mybir.dt.bfloat16
    Exp = mybir.ActivationFunctionType.Exp
    Tanh = mybir.ActivationFunctionType.Tanh
    mult = mybir.AluOpType.add  # placeholder, reassigned below
    mult = mybir.AluOpType.mult
    add = mybir.AluOpType.add

    nc = bacc.Bacc("TRN2", target_bir_lowering=False, debug=False,
                   num_devices=N_CORES)

    x_ext = nc.declare_dram_parameter("x", [B_LOC, C_IN, N], f32, isOutput=False)
    wqT_ext = nc.declare_dram_parameter("wqT", [2, 128, 256], bf16, isOutput=False)
    wkT_ext = nc.declare_dram_parameter("wkT", [2, 128, 256], bf16, isOutput=False)
    wvT_ext = nc.declare_dram_parameter("wvT", [2, 128, 512], bf16, isOutput=False)
    woT_ext = nc.declare_dram_parameter("woT", [4, 128, 256], bf16, isOutput=False)
    shq_ext = nc.declare_dram_parameter("shq", [128, 2], f32, isOutput=False)
    shk_ext = nc.declare_dram_parameter("shk", [128, 2], f32, isOutput=False)
    shv_ext = nc.declare_dram_parameter("shv", [1, 512], bf16, isOutput=False)
    ones_ext = nc.declare_dram_parameter("onesr", [1, 128], bf16, isOutput=False)
    bo_ext = nc.declare_dram_parameter("bo", [128, 2], f32, isOutput=False)
    out_ext = nc.declare_dram_parameter("out", [B_LOC, C_IN, N], f32, isOutput=True)

    from contextlib import ExitStack
    with tile.TileContext(nc) as tc, ExitStack() as ctx:
        consts = ctx.enter_context(tc.tile_pool(name="consts", bufs=1))
        vpool = ctx.enter_context(tc.tile_pool(name="vaug", bufs=1))
        xfp = ctx.enter_context(tc.tile_pool(name="xf", bufs=2))
        xbp = ctx.enter_context(tc.tile_pool(name="xb", bufs=2))
        qkp = ctx.enter_context(tc.tile_pool(name="qk", bufs=2))
        pp = ctx.enter_context(tc.tile_pool(name="pp", bufs=2))
        gp = ctx.enter_context(tc.tile_pool(name="gp", bufs=3))
        gbfp = ctx.enter_context(tc.tile_pool(name="gbf", bufs=1))
        lrp = ctx.enter_context(tc.tile_pool(name="lr", bufs=2))
        rbp = ctx.enter_context(tc.tile_pool(name="rb", bufs=2))
        osp = ctx.enter_context(tc.tile_pool(name="os", bufs=2))
        ps_big = ctx.enter_context(tc.tile_pool(name="psb", bufs=2, space="PSUM"))
        ps_av = ctx.enter_context(tc.tile_pool(name="psav", bufs=2, space="PSUM"))

        # ---- load constants ----
        wq_sb = [consts.tile([128, 256], bf16, tag=f"wq{t}", name=f"wq{t}") for t in range(2)]
        wk_sb = [consts.tile([128, 256], bf16, tag=f"wk{t}", name=f"wk{t}") for t in range(2)]
        wv_sb = [consts.tile([128, 512], bf16, tag=f"wv{t}", name=f"wv{t}") for t in range(2)]
        wo_sb = [consts.tile([128, 256], bf16, tag=f"wo{t}", name=f"wo{t}") for t in range(4)]
        shq_sb = consts.tile([128, 2], f32, tag="shq", name="shq")
        shk_sb = consts.tile([128, 2], f32, tag="shk", name="shk")
        shv_sb = consts.tile([1, 512], bf16, tag="shv", name="shv")
        ones_sb = consts.tile([1, 128], bf16, tag="ones", name="ones")
        bo_sb = consts.tile([128, 2], f32, tag="bo", name="bo")
        for t in range(2):
            nc.gpsimd.dma_start(out=wq_sb[t][:], in_=wqT_ext.ap()[t])
            nc.gpsimd.dma_start(out=wk_sb[t][:], in_=wkT_ext.ap()[t])
            nc.gpsimd.dma_start(out=wv_sb[t][:], in_=wvT_ext.ap()[t])
        for t in range(4):
            nc.gpsimd.dma_start(out=wo_sb[t][:], in_=woT_ext.ap()[t])
        nc.gpsimd.dma_start(out=shq_sb[:], in_=shq_ext.ap()[:])
        nc.gpsimd.dma_start(out=shk_sb[:], in_=shk_ext.ap()[:])
        nc.gpsimd.dma_start(out=shv_sb[:], in_=shv_ext.ap()[:])
        nc.gpsimd.dma_start(out=ones_sb[:], in_=ones_ext.ap()[:])
        nc.gpsimd.dma_start(out=bo_sb[:], in_=bo_ext.ap()[:])

        # two persistent v_aug buffers (ones columns memset once, data columns
        # rewritten per batch; av reads l from the ones column product)
        vaug = [vpool.tile([128, 64 * VSTRIDE], bf16, tag=f"vaug{i}", name=f"vaug{i}") for i in range(2)]
        nc.vector.memset(vaug[0][:], 1.0)
        nc.vector.memset(vaug[1][:], 1.0)

        for b in range(B_LOC):
            va = vaug[b % 2]
            # ---- load + cast x ----
            xf = xfp.tile([128, 2048], f32, tag="xf", name="xf")
            nc.gpsimd.dma_start(
                out=xf[:].rearrange("p (t n) -> p t n", t=2),
                in_=x_ext.ap()[b].rearrange("(t p) n -> p t n", p=128))
            xb = xbp.tile([128, 2048], bf16, tag="xb", name="xb")
            nc.vector.tensor_copy(xb[:], xf[:])

            # ---- Q/K projections (BN folded into weights; bias via evict) ----
            q_sb = [qkp.tile([128, N], bf16, tag=f"q{t}", name=f"q{t}") for t in range(2)]
            k_sb = [qkp.tile([128, N], bf16, tag=f"k{t}", name=f"k{t}") for t in range(2)]
            for (w_sb, sh_sb, dst) in ((wq_sb, shq_sb, q_sb), (wk_sb, shk_sb, k_sb)):
                for t in range(2):
                    ps = ps_big.tile([128, 1024], f32, tag="pd", name="pd")
                    for ih in range(2):
                        for kt in range(2):
                            nc.tensor.matmul(
                                ps[:, ih * 512:ih * 512 + 512],
                                w_sb[kt][:, t * 128:(t + 1) * 128],
                                xb[:, kt * 1024 + ih * 512:kt * 1024 + ih * 512 + 512],
                                start=(kt == 0), stop=(kt == 1))
                    nc.vector.tensor_scalar(
                        dst[t][:], ps[:, 0:1024],
                        sh_sb[:, t:t + 1], None, add)

            # ---- V projection, transposed ([pixel, channel]), bias via K=1 mm ----
            for half in range(4):  # j-chunks 2 at a time
                ps = ps_big.tile([128, 1024], f32, tag="pd", name="pd")
                for cq in range(2):
                    jc = half * 2 + cq
                    for kt in range(2):
                        nc.tensor.matmul(
                            ps[:, cq * 512:(cq + 1) * 512],
                            xb[:, kt * 1024 + jc * 128:kt * 1024 + jc * 128 + 128],
                            wv_sb[kt][:],
                            start=(kt == 0), stop=False)
                    nc.tensor.matmul(
                        ps[:, cq * 512:(cq + 1) * 512],
                        ones_sb[0:1, 0:128], shv_sb[0:1, :],
                        start=False, stop=True)
                for cq in range(2):
                    jc = half * 2 + cq
                    src = ps[:, cq * 512:(cq + 1) * 512].rearrange(
                        "p (h d) -> p h d", h=8)
                    dst = va[:, jc * 8 * VSTRIDE:(jc + 1) * 8 * VSTRIDE].rearrange(
                        "p (h e) -> p h e", h=8)[:, :, 0:DV]
                    nc.vector.tensor_copy(dst, src)

            # ---- attention, head pairs on distinct PE row groups ----
            gbf = []
            for p in range(4):
                h0 = 2 * p
                t_q = h0 // 4
                offs = (32 * (h0 % 4), 32 * (h0 % 4) + 32)
                P = pp.tile([128, 2 * 8192], bf16, tag="P", name="P")
                av = [ps_av.tile([128, N], f32, tag="av", name="av") for _ in range(2)]
                for jc in range(8):
                    for hi in range(2):
                        off = offs[hi]
                        pd = ps_big.tile([128, 1024], f32, tag="pd", name="pd")
                        for ih in range(2):
                            nc.tensor.matmul(
                                pd[:, ih * 512:ih * 512 + 512],
                                k_sb[t_q][off:off + 32, jc * 128:(jc + 1) * 128],
                                q_sb[t_q][off:off + 32, ih * 512:(ih + 1) * 512],
                                start=True, stop=True,
                                tile_position=(off, 0))
                        nc.scalar.activation(
                            P[:, hi * 8192 + jc * 1024:hi * 8192 + (jc + 1) * 1024],
                            pd[:], Exp)
                    for hi in range(2):
                        h = h0 + hi
                        for ih in range(2):
                            nc.tensor.matmul(
                                av[hi][0:65, ih * 512:(ih + 1) * 512],
                                va[:, jc * 8 * VSTRIDE + h * VSTRIDE:
                                   jc * 8 * VSTRIDE + h * VSTRIDE + VSTRIDE],
                                P[:, hi * 8192 + jc * 1024 + ih * 512:
                                  hi * 8192 + jc * 1024 + ih * 512 + 512],
                                start=(jc == 0), stop=(jc == 7))
                # softmax normalizer: l row -> reciprocal -> broadcast
                g = gp.tile([128, N], bf16, tag="gpre", name="gpre")
                for hi in range(2):
                    lsb = lrp.tile([1, N], f32, tag="lsb", name="lsb")
                    nc.vector.tensor_copy(lsb[0:1, :], av[hi][64:65, :])
                    rh = lrp.tile([1, N], f32, tag="rh", name="rh")
                    # approx recip needs in/out base partitions equal
                    nc.vector.reciprocal_approx_fast(rh[0:1, :], lsb[0:1, :])
                    R = rbp.tile([64, N], f32, tag="R", name="R")
                    nc.gpsimd.partition_broadcast(R[:], rh[0:1, :])
                    nc.vector.tensor_tensor(
                        g[hi * 64:hi * 64 + 64, :], av[hi][0:64, :], R[:], mult)
                # gelu via tanh form (tanh shares the exp ACT table set;
                # the 0.5 factor is folded into the output weights):
                # gelu(x)/0.5 = x * (1 + tanh(c*x + c*0.044715*x^3))
                GC = 0.7978845608028654
                GA = GC * 0.044715
                t1 = lrp.tile([128, N], bf16, tag="t1", name="t1")
                nc.vector.scalar_tensor_tensor(t1[:], g[:], GA, g[:], mult, mult)
                t2 = lrp.tile([128, N], bf16, tag="t2", name="t2")
                nc.vector.scalar_tensor_tensor(t2[:], t1[:], GC, g[:], add, mult)
                t3 = lrp.tile([128, N], bf16, tag="t3", name="t3")
                nc.scalar.activation(t3[:], t2[:], Tanh)
                gt = gbfp.tile([128, N], bf16, tag=f"gbf{p}", name=f"gbf{p}")
                gbf.append(gt)
                nc.vector.scalar_tensor_tensor(gt[:], t3[:], 1.0, g[:], add, mult)

            # ---- output projection + folded BN bias ----
            ps_o = ps_av.tile([128, N], f32, tag="av", name="av")
            ps_o2 = ps_av.tile([128, N], f32, tag="av", name="av")
            for ot, pso in ((0, ps_o), (1, ps_o2)):
                for kt in range(4):
                    for ih in range(2):
                        nc.tensor.matmul(
                            pso[:, ih * 512:(ih + 1) * 512],
                            wo_sb[kt][:, ot * 128:(ot + 1) * 128],
                            gbf[kt][:, ih * 512:(ih + 1) * 512],
                            start=(kt == 0), stop=(kt == 3))
                osb = osp.tile([128, N], f32, tag="osb", name="osb")
                nc.vector.tensor_scalar(osb[:], pso[:, 0:N],
                                        bo_sb[:, ot:ot + 1], None, add)
                nc.gpsimd.dma_start(
                    out=out_ext.ap()[b, ot * 128:(ot + 1) * 128, :], in_=osb[:])

    nc.compile()
    return nc


def _get_nc():
    if 'nc' not in _cache:
        _cache['nc'] = _build()
    return _cache['nc']


def _fold_weights(inputs):
    """Fold BatchNorms (+ attention scale) into conv weights, host-side."""
    f8 = {k: np.asarray(v, np.float64) for k, v in inputs.items()}
    scale = DK ** -0.5

    def fold(w, g, b, m, v, extra=1.0):
        inv = g / np.sqrt(v + EPS)
        return w * inv[:, None] * extra, (b - m * inv) * extra

    wq_e, shq = fold(f8['wq'], f8['gq'], f8['bq'], f8['mq'], f8['vq'], scale)
    wk_e, shk = fold(f8['wk'], f8['gk'], f8['bk'], f8['mk'], f8['vk'])
    wv_e, shv = fold(f8['wv'], f8['gv'], f8['bv'], f8['mv'], f8['vv'])
    inv_o = f8['go'] / np.sqrt(f8['vo'] + EPS)
    # the 0.5 of the tanh-form gelu is folded in here
    wo_e = f8['wo'] * inv_o[:, None] * 0.5
    bo = inv_o * f8['b_out'] + (f8['be_o'] - f8['mo'] * inv_o)

    bf = ml_dtypes.bfloat16
    return {
        'wqT': np.ascontiguousarray(wq_e.T).reshape(2, 128, 256).astype(bf),
        'wkT': np.ascontiguousarray(wk_e.T).reshape(2, 128, 256).astype(bf),
        'wvT': np.ascontiguousarray(wv_e.T).reshape(2, 128, 512).astype(bf),
        'woT': np.ascontiguousarray(wo_e.T).reshape(4, 128, 256).astype(bf),
        'shq': np.ascontiguousarray(shq.reshape(2, 128).T).astype(np.float32),
        'shk': np.ascontiguousarray(shk.reshape(2, 128).T).astype(np.float32),
        'shv': shv.reshape(1, 512).astype(bf),
        'onesr': np.ones((1, 128), bf),
        'bo': np.ascontiguousarray(bo.reshape(2, 128).T).astype(np.float32),
    }


def kernel_run(inputs, trace=False, trace_kwargs=None):
    from concourse.bass_utils import run_bass_kernel_spmd
    nc = _get_nc()
    consts = _fold_weights(inputs)
    x = np.asarray(inputs['x'], np.float32).reshape(B_TOT, C_IN, N)
    in_maps = []
    for c in range(N_CORES):
        m = dict(consts)
        m['x'] = np.ascontiguousarray(x[c * B_LOC:(c + 1) * B_LOC])
        in_maps.append(m)
    res = run_bass_kernel_spmd(nc, in_maps, core_ids=list(range(N_CORES)),
                               trace=trace, **(trace_kwargs or {}))
    out = np.concatenate([res.results[c]['out'] for c in range(N_CORES)], axis=0)
    return out.reshape(B_TOT, C_IN, 32, 32), res


def kernel(**inputs) -> np.ndarray:
    out, _ = kernel_run(inputs, trace=False)
    return out


# revision 25
# speedup vs baseline: 1.7035x; 1.1126x over previous
"""Trainium2 Bass kernel for nn_Attention_24215025615017.

8-head spatial attention block (1x1-conv QKV projections with folded BatchNorm,
transposed-softmax attention, exact GELU, output 1x1 conv with folded BN).
Data-parallel over batch: B=32 sharded as 4 batches on each of 8 NeuronCores.

Self-contained: hardcodes shapes/sharding; builds + caches one SPMD Bacc graph.
"""

import sys
import numpy as np

if '/opt/trn_rl_repo' not in sys.path:
    sys.path.insert(0, '/opt/trn_rl_repo')
_a = sys.modules.get('antenv')
if _a is not None and '_ro' in getattr(_a, '__file__', ''):
    # purge the read-only copy so antenv resolves to /opt/trn_rl_repo
    for _m in list(sys.modules):
        if _m == 'antenv' or _m.startswith('antenv.'):
            del sys.modules[_m]

import ml_dtypes

EPS = 1e-5
HEADS = 8
DK = 32
DV = 64
B_TOT = 32
N_CORES = 8
B_LOC = B_TOT // N_CORES  # 4 batches per core
C_IN = 256                # input channels
C_V = 512                 # v channels (h*dv)
N = 1024                  # pixels (32*32)
VSTRIDE = DV + 1          # v_aug block: 64 data cols + ones col

_cache = {}


def _build():
    import concourse.bass as bass
    import concourse.tile as tile
    from concourse import bacc, mybir

    f32 = mybir.dt.float32
    bf16 = mybir.dt.bfloat16
    Exp = mybir.ActivationFunctionType.Exp
    Tanh = mybir.ActivationFunctionType.Tanh
    mult = mybir.AluOpType.add  # placeholder, reassigned below
    mult = mybir.AluOpType.mult
    add = mybir.AluOpType.add

    nc = bacc.Bacc("TRN2", target_bir_lowering=False, debug=False,
                   num_devices=N_CORES)

    x_ext = nc.declare_dram_parameter("x", [B_LOC, C_IN, N], f32, isOutput=False)
    wqT_ext = nc.declare_dram_parameter("wqT", [2, 128, 256], bf16, isOutput=False)
    wkT_ext = nc.declare_dram_parameter("wkT", [2, 128, 256], bf16, isOutput=False)
    wvT_ext = nc.declare_dram_parameter("wvT", [2, 128, 512], bf16, isOutput=False)
    woT_ext = nc.declare_dram_parameter("woT", [4, 128, 256], bf16, isOutput=False)
    shq_ext = nc.declare_dram_parameter("shq", [128, 2], f32, isOutput=False)
    shk_ext = nc.declare_dram_parameter("shk", [128, 2], f32, isOutput=False)
    shv_ext = nc.declare_dram_parameter("shv", [1, 512], bf16, isOutput=False)
    ones_ext = nc.declare_dram_parameter("onesr", [1, 128], bf16, isOutput=False)
    bo_ext = nc.declare_dram_parameter("bo", [128, 2], f32, isOutput=False)
    out_ext = nc.declare_dram_parameter("out", [B_LOC, C_IN, N], f32, isOutput=True)

    from contextlib import ExitStack
    with tile.TileContext(nc) as tc, ExitStack() as ctx:
        consts = ctx.enter_context(tc.tile_pool(name="consts", bufs=1))
        vpool = ctx.enter_context(tc.tile_pool(name="vaug", bufs=1))
        xfp = ctx.enter_context(tc.tile_pool(name="xf", bufs=2))
        xbp = ctx.enter_context(tc.tile_pool(name="xb", bufs=2))
        qkp = ctx.enter_context(tc.tile_pool(name="qk", bufs=2))
        pp = ctx.enter_context(tc.tile_pool(name="pp", bufs=2))
        gp = ctx.enter_context(tc.tile_pool(name="gp", bufs=3))
        gbfp = ctx.enter_context(tc.tile_pool(name="gbf", bufs=1))
        lrp = ctx.enter_context(tc.tile_pool(name="lr", bufs=2))
        rbp = ctx.enter_context(tc.tile_pool(name="rb", bufs=2))
        osp = ctx.enter_context(tc.tile_pool(name="os", bufs=2))
        ps_big = ctx.enter_context(tc.tile_pool(name="psb", bufs=2, space="PSUM"))
        ps_av = ctx.enter_context(tc.tile_pool(name="psav", bufs=2, space="PSUM"))

        # ---- load constants ----
        wq_sb = [consts.tile([128, 256], bf16, tag=f"wq{t}", name=f"wq{t}") for t in range(2)]
        wk_sb = [consts.tile([128, 256], bf16, tag=f"wk{t}", name=f"wk{t}") for t in range(2)]
        wv_sb = [consts.tile([128, 512], bf16, tag=f"wv{t}", name=f"wv{t}") for t in range(2)]
        wo_sb = [consts.tile([128, 256], bf16, tag=f"wo{t}", name=f"wo{t}") for t in range(4)]
        shq_sb = consts.tile([128, 2], f32, tag="shq", name="shq")
        shk_sb = consts.tile([128, 2], f32, tag="shk", name="shk")
        shv_sb = consts.tile([1, 512], bf16, tag="shv", name="shv")
        ones_sb = consts.tile([1, 128], bf16, tag="ones", name="ones")
        bo_sb = consts.tile([128, 2], f32, tag="bo", name="bo")
        for t in range(2):
            nc.gpsimd.dma_start(out=wq_sb[t][:], in_=wqT_ext.ap()[t])
            nc.gpsimd.dma_start(out=wk_sb[t][:], in_=wkT_ext.ap()[t])
            nc.gpsimd.dma_start(out=wv_sb[t][:], in_=wvT_ext.ap()[t])
        for t in range(4):
            nc.gpsimd.dma_start(out=wo_sb[t][:], in_=woT_ext.ap()[t])
        nc.gpsimd.dma_start(out=shq_sb[:], in_=shq_ext.ap()[:])
        nc.gpsimd.dma_start(out=shk_sb[:], in_=shk_ext.ap()[:])
        nc.gpsimd.dma_start(out=shv_sb[:], in_=shv_ext.ap()[:])
        nc.gpsimd.dma_start(out=ones_sb[:], in_=ones_ext.ap()[:])
        nc.gpsimd.dma_start(out=bo_sb[:], in_=bo_ext.ap()[:])

        # two persistent v_aug buffers (ones columns memset once, data columns
        # rewritten per batch; av reads l from the ones column product)
        vaug = [vpool.tile([128, 64 * VSTRIDE], bf16, tag=f"vaug{i}", name=f"vaug{i}") for i in range(2)]
        nc.vector.memset(vaug[0][:], 1.0)
        nc.vector.memset(vaug[1][:], 1.0)

        for b in range(B_LOC):
            va = vaug[b % 2]
            # ---- load + cast x ----
            xf = xfp.tile([128, 2048], f32, tag="xf", name="xf")
            nc.gpsimd.dma_start(
                out=xf[:].rearrange("p (t n) -> p t n", t=2),
                in_=x_ext.ap()[b].rearrange("(t p) n -> p t n", p=128))
            xb = xbp.tile([128, 2048], bf16, tag="xb", name="xb")
            nc.vector.tensor_copy(xb[:], xf[:])

            # ---- Q/K projections (BN folded into weights; bias via evict) ----
            q_sb = [qkp.tile([128, N], bf16, tag=f"q{t}", name=f"q{t}") for t in range(2)]
            k_sb = [qkp.tile([128, N], bf16, tag=f"k{t}", name=f"k{t}") for t in range(2)]
            for (w_sb, sh_sb, dst) in ((wq_sb, shq_sb, q_sb), (wk_sb, shk_sb, k_sb)):
                for t in range(2):
                    ps = ps_big.tile([128, 1024], f32, tag="pd", name="pd")
                    for ih in range(2):
                        for kt in range(2):
                            nc.tensor.matmul(
                                ps[:, ih * 512:ih * 512 + 512],
                                w_sb[kt][:, t * 128:(t + 1) * 128],
                                xb[:, kt * 1024 + ih * 512:kt * 1024 + ih * 512 + 512],
                                start=(kt == 0), stop=(kt == 1))
                    nc.vector.tensor_scalar(
                        dst[t][:], ps[:, 0:1024],
                        sh_sb[:, t:t + 1], None, add)

            # ---- V projection, transposed ([pixel, channel]), bias via K=1 mm ----
            for half in range(4):  # j-chunks 2 at a time
                ps = ps_big.tile([128, 1024], f32, tag="pd", name="pd")
                for cq in range(2):
                    jc = half * 2 + cq
                    for kt in range(2):
                        nc.tensor.matmul(
                            ps[:, cq * 512:(cq + 1) * 512],
                            xb[:, kt * 1024 + jc * 128:kt * 1024 + jc * 128 + 128],
                            wv_sb[kt][:],
                            start=(kt == 0), stop=False)
                    nc.tensor.matmul(
                        ps[:, cq * 512:(cq + 1) * 512],
                        ones_sb[0:1, 0:128], shv_sb[0:1, :],
                        start=False, stop=True)
                for cq in range(2):
                    jc = half * 2 + cq
                    src = ps[:, cq * 512:(cq + 1) * 512].rearrange(
                        "p (h d) -> p h d", h=8)
                    dst = va[:, jc * 8 * VSTRIDE:(jc + 1) * 8 * VSTRIDE].rearrange(
                        "p (h e) -> p h e", h=8)[:, :, 0:DV]
                    nc.vector.tensor_copy(dst, src)

            # ---- attention, head pairs on distinct PE row groups ----
            gbf = []
            for p in range(4):
                h0 = 2 * p
                t_q = h0 // 4
                offs = (32 * (h0 % 4), 32 * (h0 % 4) + 32)
                P = pp.tile([128, 2 * 8192], bf16, tag="P", name="P")
                av = [ps_av.tile([128, N], f32, tag="av", name="av") for _ in range(2)]
                def do_av(jc):
                    for hi in range(2):
                        h = h0 + hi
                        for ih in range(2):
                            nc.tensor.matmul(
                                av[hi][0:65, ih * 512:(ih + 1) * 512],
                                va[:, jc * 8 * VSTRIDE + h * VSTRIDE:
                                   jc * 8 * VSTRIDE + h * VSTRIDE + VSTRIDE],
                                P[:, hi * 8192 + jc * 1024 + ih * 512:
                                  hi * 8192 + jc * 1024 + ih * 512 + 512],
                                start=(jc == 0), stop=(jc == 7))

                # software-pipelined: av of chunk jc-1 issues after dots of
                # chunk jc, so PE never waits on the exp of the same chunk
                for jc in range(8):
                    for hi in range(2):
                        off = offs[hi]
                        pd = ps_big.tile([128, 1024], f32, tag="pd", name="pd")
                        for ih in range(2):
                            nc.tensor.matmul(
                                pd[:, ih * 512:ih * 512 + 512],
                                k_sb[t_q][off:off + 32, jc * 128:(jc + 1) * 128],
                                q_sb[t_q][off:off + 32, ih * 512:(ih + 1) * 512],
                                start=True, stop=True,
                                tile_position=(off, 0))
                        nc.scalar.activation(
                            P[:, hi * 8192 + jc * 1024:hi * 8192 + (jc + 1) * 1024],
                            pd[:], Exp)
                    if jc >= 1:
                        do_av(jc - 1)
                do_av(7)
                # softmax normalizer: l row -> reciprocal -> broadcast
                g = gp.tile([128, N], bf16, tag="gpre", name="gpre")
                for hi in range(2):
                    lsb = lrp.tile([1, N], f32, tag="lsb", name="lsb")
                    nc.vector.tensor_copy(lsb[0:1, :], av[hi][64:65, :])
                    rh = lrp.tile([1, N], f32, tag="rh", name="rh")
                    # approx recip needs in/out base partitions equal
                    nc.vector.reciprocal_approx_fast(rh[0:1, :], lsb[0:1, :])
                    R = rbp.tile([64, N], f32, tag="R", name="R")
                    nc.gpsimd.partition_broadcast(R[:], rh[0:1, :])
                    nc.vector.tensor_tensor(
                        g[hi * 64:hi * 64 + 64, :], av[hi][0:64, :], R[:], mult)
                # gelu via tanh form (tanh shares the exp ACT table set;
                # the 0.5 factor is folded into the output weights):
                # gelu(x)/0.5 = x * (1 + tanh(c*x + c*0.044715*x^3))
                GC = 0.7978845608028654
                GA = GC * 0.044715
                t1 = lrp.tile([128, N], bf16, tag="t1", name="t1")
                nc.vector.scalar_tensor_tensor(t1[:], g[:], GA, g[:], mult, mult)
                t2 = lrp.tile([128, N], bf16, tag="t2", name="t2")
                nc.vector.scalar_tensor_tensor(t2[:], t1[:], GC, g[:], add, mult)
                t3 = lrp.tile([128, N], bf16, tag="t3", name="t3")
                nc.scalar.activation(t3[:], t2[:], Tanh)
                gt = gbfp.tile([128, N], bf16, tag=f"gbf{p}", name=f"gbf{p}")
                gbf.append(gt)
                nc.vector.scalar_tensor_tensor(gt[:], t3[:], 1.0, g[:], add, mult)

            # ---- output projection + folded BN bias ----
            ps_o = ps_av.tile([128, N], f32, tag="av", name="av")
            ps_o2 = ps_av.tile([128, N], f32, tag="av", name="av")
            for ot, pso in ((0, ps_o), (1, ps_o2)):
                for kt in range(4):
                    for ih in range(2):
                        nc.tensor.matmul(
                            pso[:, ih * 512:(ih + 1) * 512],
                            wo_sb[kt][:, ot * 128:(ot + 1) * 128],
                            gbf[kt][:, ih * 512:(ih + 1) * 512],
                            start=(kt == 0), stop=(kt == 3))
                osb = osp.tile([128, N], f32, tag="osb", name="osb")
                nc.vector.tensor_scalar(osb[:], pso[:, 0:N],
                                        bo_sb[:, ot:ot + 1], None, add)
                nc.gpsimd.dma_start(
                    out=out_ext.ap()[b, ot * 128:(ot + 1) * 128, :], in_=osb[:])

    nc.compile()
    return nc


def _get_nc():
    if 'nc' not in _cache:
        _cache['nc'] = _build()
    return _cache['nc']


def _fold_weights(inputs):
    """Fold BatchNorms (+ attention scale) into conv weights, host-side."""
    f8 = {k: np.asarray(v, np.float64) for k, v in inputs.items()}
    scale = DK ** -0.5

    def fold(w, g, b, m, v, extra=1.0):
        inv = g / np.sqrt(v + EPS)
        return w * inv[:, None] * extra, (b - m * inv) * extra

    wq_e, shq = fold(f8['wq'], f8['gq'], f8['bq'], f8['mq'], f8['vq'], scale)
    wk_e, shk = fold(f8['wk'], f8['gk'], f8['bk'], f8['mk'], f8['vk'])
    wv_e, shv = fold(f8['wv'], f8['gv'], f8['bv'], f8['mv'], f8['vv'])
    inv_o = f8['go'] / np.sqrt(f8['vo'] + EPS)
    # the 0.5 of the tanh-form gelu is folded in here
    wo_e = f8['wo'] * inv_o[:, None] * 0.5
    bo = inv_o * f8['b_out'] + (f8['be_o'] - f8['mo'] * inv_o)

    bf = ml_dtypes.bfloat16
    return {
        'wqT': np.ascontiguousarray(wq_e.T).reshape(2, 128, 256).astype(bf),
        'wkT': np.ascontiguousarray(wk_e.T).reshape(2, 128, 256).astype(bf),
        'wvT': np.ascontiguousarray(wv_e.T).reshape(2, 128, 512).astype(bf),
        'woT': np.ascontiguousarray(wo_e.T).reshape(4, 128, 256).astype(bf),
        'shq': np.ascontiguousarray(shq.reshape(2, 128).T).astype(np.float32),
        'shk': np.ascontiguousarray(shk.reshape(2, 128).T).astype(np.float32),
        'shv': shv.reshape(1, 512).astype(bf),
        'onesr': np.ones((1, 128), bf),
        'bo': np.ascontiguousarray(bo.reshape(2, 128).T).astype(np.float32),
    }


def kernel_run(inputs, trace=False, trace_kwargs=None):
    from concourse.bass_utils import run_bass_kernel_spmd
    nc = _get_nc()
    consts = _fold_weights(inputs)
    x = np.asarray(inputs['x'], np.float32).reshape(B_TOT, C_IN, N)
    in_maps = []
    for c in range(N_CORES):
        m = dict(consts)
        m['x'] = np.ascontiguousarray(x[c * B_LOC:(c + 1) * B_LOC])
        in_maps.append(m)
    res = run_bass_kernel_spmd(nc, in_maps, core_ids=list(range(N_CORES)),
                               trace=trace, **(trace_kwargs or {}))
    out = np.concatenate([res.results[c]['out'] for c in range(N_CORES)], axis=0)
    return out.reshape(B_TOT, C_IN, 32, 32), res


def kernel(**inputs) -> np.ndarray:
    out, _ = kernel_run(inputs, trace=False)
    return out
